# revision 20
# baseline (speedup 1.0000x reference)
"""Trainium2 Bass kernel for nn_DiscreteQKTRBlock (sparse 3x3x3 neighborhood
attention with a discrete codebook).

Strategy (data-parallel over points, 8 cores, replicated tables):

Algebraic collapse: dq[i] = codebook * choice[i], so the per-offset score is
s[k,i] = ||codebook||^2 * choice[i] * choice[nbr[k,i]] - a scalar product.

Phases per core (own points NSH=12544 = 98 tiles; tables PAD=100352 rows):
  Y      build Yf[(j,k)] = x[j] @ Wq_k  (fp16, desc-optimized p-major layout)
  A      per own tile, per valid slot: 128-row indirect gather from Yf with
         CCE-add accumulation -> q, then BN/relu + choice head -> strip.
         (HW indirect DMA processes exactly one index per partition.)
  TV     (overlapped under A) build Tv[j] = [v fp16 128 | choice f32 | xyz
         fp16] in 512B rows; positional encoding is NOT applied: coords ride
         in the row, the softmax-weighted sum aggregates them (sum w = 1), and
         W_pos / b_pos are folded into the output projection / bias.
  AG     transpose strip to point-major, AllGather choice, patch the choice
         f32 column into all Tv rows with one strided write.
  C      windowed int16 dma_gather of Tv rows: the 100352-row table is
         addressed in four <32768-row windows; each point's valid slots are
         compacted per (tile, window) in per-partition SCRAMBLED order (the
         softmax and the weighted sum are order-agnostic per partition; the
         score bias is host-staged in the same order and choice rides in the
         gathered row, so scrambling is benign).  One dma_gather call per
         (3-tile group, window) replaces ~40 per-slot SWDGE calls.
         Then masked softmax in f32, fp16 weighted aggregation (values +
         coords together), folded output projection, relu, residual.
"""
import sys
sys.path.insert(0, "/opt/trn_rl_repo")
import numpy as np
import ml_dtypes

from concourse import bass, bacc, mybir
import concourse.tile as tile
from concourse.bass_utils import run_bass_kernel_spmd
from concourse.masks import make_identity

F32 = mybir.dt.float32
FP16 = mybir.dt.float16
I32 = mybir.dt.int32
I16 = mybir.dt.int16
AF = mybir.ActivationFunctionType
ALU = mybir.AluOpType

N = 100000
P = 128
VEC = 16
K = 27
NEG = -1e9
NCORE = 8
NSH = 12544                # points per core (98 tiles of 128)
TO = NSH // P              # 98 own tiles
PAD = NCORE * NSH          # 100352 table rows
TA = PAD // P              # 784 table tiles
Z = N                      # zero-row index for masked/padded neighbors
NW = 2                     # dma_gather windows over the table
WR = PAD // NW             # 50176 rows per window (signed-int16 centered base)
WC = WR // 2               # centered base offset within a window
ES = 256                   # Tv row elems (fp16): v 128 | choice f32 | xyz | pad
G = 8                      # build tiles per group
NG = TA // G
GC = 2                     # own tiles per C gather group

_CACHE = {}


def _build_nc(kts, Js):
    off = [0]
    for w in kts:
        off.append(off[-1] + int(w))
    SKT = off[-1]
    # C-phase position bookkeeping (all build-time constants)
    posoff = [0]            # per tile: start of its positions
    for t in range(TO):
        posoff.append(posoff[-1] + sum(Js[t]))
    TOTPOS = posoff[-1]
    groups = [list(range(g, min(g + GC, TO))) for g in range(0, TO, GC)]
    # per (group, window): column count and idxC offset (in positions)
    call_cols = {}
    cum = 0
    for gi, gts in enumerate(groups):
        for w in range(NW):
            c = sum(Js[t][w] for t in gts)
            if c > 0:
                c += 1          # trailing all-pad column (negative-trim guard)
            call_cols[(gi, w)] = (cum, c)
            cum += c
    NCOLS = cum

    nc = bacc.Bacc(num_devices=NCORE, dynamic_dma_scratch_size=32768)

    # ---------------- inputs ----------------
    xT16 = nc.declare_dram_parameter("xT16", [P, PAD], FP16, isOutput=False)
    c3g_in = nc.declare_dram_parameter("c3g", [P, TA * 3], FP16, isOutput=False)
    wqv_in = nc.declare_dram_parameter("wqv", [P, 560], FP16, isOutput=False)
    vb_in = nc.declare_dram_parameter("vb", [1, P], FP16, isOutput=False)
    qg_in = nc.declare_dram_parameter("qg", [VEC, 1], F32, isOutput=False)
    qb_in = nc.declare_dram_parameter("qb", [VEC, 1], F32, isOutput=False)
    wcc_in = nc.declare_dram_parameter("wcc", [VEC, P], FP16, isOutput=False)
    bch_in = nc.declare_dram_parameter("bch", [1, P], FP16, isOutput=False)
    wo_in = nc.declare_dram_parameter("wo", [P, P], FP16, isOutput=False)
    wpor_in = nc.declare_dram_parameter("wpor", [3, P], FP16, isOutput=False)
    ob_in = nc.declare_dram_parameter("ob", [P, 1], F32, isOutput=False)
    idxa_in = nc.declare_dram_parameter("idxa_p", [P, SKT], I32, isOutput=False)
    idxr_in = nc.declare_dram_parameter("idxr_p", [P, SKT], I32, isOutput=False)
    bias_in = nc.declare_dram_parameter("bias_p", [P, SKT], F32,
                                        isOutput=False)
    xoT_in = nc.declare_dram_parameter("xoT", [P, NSH], FP16, isOutput=False)

    outT = nc.declare_dram_parameter("outT", [P, NSH], F32, isOutput=True)

    with tile.TileContext(nc) as tc:
        with tc.tile_pool(name="const", bufs=1) as cpool, \
             tc.tile_pool(name="dram", bufs=1, space="DRAM") as dpool:

            # ---------------- resident constants ----------------
            wqv_sb = cpool.tile([P, 560], FP16)
            nc.scalar.dma_start(out=wqv_sb[:], in_=wqv_in[:, :])
            vb_sb = cpool.tile([1, P], FP16)
            nc.scalar.dma_start(out=vb_sb[:], in_=vb_in[:, :])
            qg_sb = cpool.tile([VEC, 1], F32)
            nc.scalar.dma_start(out=qg_sb[:], in_=qg_in[:, :])
            qb_sb = cpool.tile([VEC, 1], F32)
            nc.scalar.dma_start(out=qb_sb[:], in_=qb_in[:, :])
            wcc_sb = cpool.tile([VEC, P], FP16)
            nc.scalar.dma_start(out=wcc_sb[:], in_=wcc_in[:, :])
            bch_sb = cpool.tile([1, P], FP16)
            nc.scalar.dma_start(out=bch_sb[:], in_=bch_in[:, :])
            wo_sb = cpool.tile([P, P], FP16)
            nc.scalar.dma_start(out=wo_sb[:], in_=wo_in[:, :])
            wpor_sb = cpool.tile([3, P], FP16)
            nc.scalar.dma_start(out=wpor_sb[:], in_=wpor_in[:, :])
            ob_sb = cpool.tile([P, 1], F32)
            nc.scalar.dma_start(out=ob_sb[:], in_=ob_in[:, :])
            idxa_sb = cpool.tile([P, SKT], I32)
            nc.sync.dma_start(out=idxa_sb[:], in_=idxa_in[:, :])
            idxr_sb = cpool.tile([P, SKT], I32)
            nc.sync.dma_start(out=idxr_sb[:], in_=idxr_in[:, :])
            bias_sb = cpool.tile([P, SKT], F32)
            nc.sync.dma_start(out=bias_sb[:], in_=bias_in[:, :])
            c3g_sb = cpool.tile([P, TA * 3], FP16)
            nc.sync.dma_start(out=c3g_sb[:], in_=c3g_in[:, :])

            ident16 = cpool.tile([P, P], FP16)
            make_identity(nc, ident16[:])
            ident32 = cpool.tile([P, P], F32)
            make_identity(nc, ident32[:])
            ones16 = cpool.tile([1, P], FP16)
            nc.vector.memset(ones16[:], 1.0)

            strip = cpool.tile([P, TO], F32)
            st_sb = cpool.tile([P, P], F32)
            ch_sb = cpool.tile([P, TA], F32)     # allgathered choice, p-major

            # ---------------- DRAM tables ----------------
            Yf = dpool.tile([PAD * K, VEC], FP16)
            Tv = dpool.tile([PAD, ES], FP16)
            cc_in = dpool.tile([NSH, 1], F32)
            AGout = dpool.tile([NCORE, NSH, 1], F32, addr_space="Shared")

            from contextlib import ExitStack
            _stk = ExitStack()
            xpool = _stk.enter_context(tc.tile_pool(name="b_x", bufs=3))
            ypool = _stk.enter_context(tc.tile_pool(name="b_y", bufs=2))
            tpool = _stk.enter_context(tc.tile_pool(name="b_t", bufs=2))
            ps1 = _stk.enter_context(tc.tile_pool(name="ps1", bufs=2, space="PSUM"))
            ps2 = _stk.enter_context(tc.tile_pool(name="ps2", bufs=4, space="PSUM"))
            apool = _stk.enter_context(tc.tile_pool(name="a_g", bufs=3))
            qfpool = _stk.enter_context(tc.tile_pool(name="a_qf", bufs=2))
            scrpool = _stk.enter_context(tc.tile_pool(name="a_scr", bufs=2))
            gpools = [_stk.enter_context(tc.tile_pool(name=f"c_g{w}", bufs=2))
                      for w in range(NW)]
            spool = _stk.enter_context(tc.tile_pool(name="c_s", bufs=3))
            accpool = _stk.enter_context(tc.tile_pool(name="c_acc", bufs=2))
            vtpool = _stk.enter_context(tc.tile_pool(name="c_vt", bufs=2))
            opool = _stk.enter_context(tc.tile_pool(name="c_o", bufs=2))
            rpool = _stk.enter_context(tc.tile_pool(name="c_r", bufs=3))

            # ---------------- phase Y: Yf = x @ Wq (p-major blocks) --------
            with nc.named_scope("phaseY"):
                for g in range(NG):
                    xg = xpool.tile([P, G * P], FP16, tag="xg")
                    nc.sync.dma_start(out=xg[:],
                                      in_=xT16[:, g * G * P:(g + 1) * G * P])
                    y8 = ypool.tile([P, G * 432], FP16, tag="y8")
                    for i in range(G):
                        y_ps = ps1.tile([P, 432], F32, tag="p1")
                        nc.tensor.matmul(out=y_ps[:],
                                         lhsT=xg[:, i * P:(i + 1) * P],
                                         rhs=wqv_sb[:, 0:432],
                                         start=True, stop=True)
                        ysl = y8[:, i * 432:(i + 1) * 432]
                        if i % 2 == 0:
                            nc.vector.tensor_copy(out=ysl, in_=y_ps[:])
                        else:
                            nc.scalar.activation(out=ysl, in_=y_ps[:],
                                                 func=AF.Copy)
                    # partition p holds Y blocks for table rows (t*128+p),
                    # stored p-major: block index p*TA + t  -> one descriptor
                    # per partition per group
                    ydst = bass.AP(Yf.tensor, Yf[:].offset + g * G * 432,
                                   [(TA * 432, P), (432, G), (1, 432)])
                    nc.scalar.dma_start(out=ydst, in_=y8[:])

            # ---------------- phase TV: value table (overlaps phase A) -----
            def emit_tv_group(g):
                if True:
                    xg = xpool.tile([P, G * P], FP16, tag="xg")
                    nc.sync.dma_start(out=xg[:],
                                      in_=xT16[:, g * G * P:(g + 1) * G * P])
                    t8 = tpool.tile([P, G * ES], FP16, tag="t8")
                    for i in range(G):
                        v_ps = ps2.tile([P, P], F32, tag="p2")
                        nc.tensor.matmul(out=v_ps[:],
                                         lhsT=xg[:, i * P:(i + 1) * P],
                                         rhs=wqv_sb[:, 432:560],
                                         start=True, stop=False)
                        nc.tensor.matmul(out=v_ps[:], lhsT=ones16[:],
                                         rhs=vb_sb[:], start=False, stop=True)
                        tsl = t8[:, i * ES:i * ES + P]
                        if i % 2 == 0:
                            nc.scalar.activation(out=tsl, in_=v_ps[:],
                                                 func=AF.Relu)
                        else:
                            nc.vector.tensor_scalar_max(out=tsl, in0=v_ps[:],
                                                        scalar1=0.0)
                        # coords into row elems 130..132
                        gi = g * G + i
                        nc.vector.tensor_copy(
                            out=t8[:, i * ES + 130:i * ES + 133],
                            in_=c3g_sb[:, gi * 3:(gi + 1) * 3])
                    tdst = bass.AP(Tv.tensor, Tv[:].offset + g * G * P * ES,
                                   [(ES, P), (ES * P, G), (1, ES)])
                    nc.sync.dma_start(out=tdst, in_=t8[:])

            # ---------------- phase A: q + choice (per-slot gathers) -------
            def emit_a_tile(t):
                if True:
                    w = kts[t]
                    qacc = apool.tile([P, VEC], FP16, tag="qacc")
                    for s in range(w):
                        nc.gpsimd.indirect_dma_start(
                            out=qacc[:], out_offset=None, in_=Yf[:, :],
                            in_offset=bass.IndirectOffsetOnAxis(
                                ap=idxa_sb[:, off[t] + s:off[t] + s + 1],
                                axis=0),
                            compute_op=(ALU.bypass if s == 0 else ALU.add))
                    q_ps = ps2.tile([VEC, P], FP16, tag="p2",
                                    padded_shape=[P, P])
                    nc.tensor.transpose(out=q_ps[:], in_=qacc[:],
                                        identity=ident16[:])
                    qf = qfpool.tile([VEC, P], FP16, tag="qf")
                    nc.scalar.activation(out=qf[:], in_=q_ps[:], func=AF.Relu,
                                         bias=qb_sb[:, 0:1],
                                         scale=qg_sb[:, 0:1])
                    t_ps = ps1.tile([P, P], F32, tag="p1")
                    nc.tensor.matmul(out=t_ps[:], lhsT=qf[:], rhs=wcc_sb[:],
                                     start=True, stop=False)
                    nc.tensor.matmul(out=t_ps[:], lhsT=ones16[:],
                                     rhs=bch_sb[:], start=False, stop=True)
                    scr = scrpool.tile([P, P], F32, tag="scr")
                    nc.scalar.activation(out=scr[:], in_=t_ps[:], func=AF.Relu,
                                         accum_out=strip[:, t:t + 1])

            def emit_atv():
                for i in range(max(NG, TO)):
                    if i < NG:
                        emit_tv_group(i)
                    if i < TO:
                        emit_a_tile(i)

            # interleave TV groups and A tiles so the in-order PE/ACT/DVE
            # queues don't head-of-line block TV behind A's gather waits
            with nc.named_scope("phaseATV"):
                emit_atv()

            # ---------------- AllGather choice + patch into Tv -------------
            with nc.named_scope("gather_choice"):
                st_ps = ps1.tile([P, P], F32, tag="p1")
                nc.tensor.transpose(out=st_ps[0:TO, :], in_=strip[:],
                                    identity=ident32[:])
                nc.vector.tensor_copy(out=st_sb[0:TO, :], in_=st_ps[0:TO, :])
                ccdst = bass.AP(cc_in.tensor, cc_in[:].offset, [(P, TO), (1, P)])
                nc.sync.dma_start(out=ccdst, in_=st_sb[0:TO, :])
                nc.gpsimd.collective_compute(
                    "AllGather", ALU.bypass,
                    replica_groups=[list(range(NCORE))],
                    ins=[cc_in.opt()], outs=[AGout.opt()])
                # load p-major: ch_sb[p, t] = choice[table row p*TA + t]
                agf = AGout[:, :, :].rearrange("r n v -> (r n v)")
                nc.sync.dma_start(
                    out=ch_sb[:], in_=agf.rearrange("(p t) -> p t", p=P))
                # patch: choice f32 lives at fp16-elems [128,130) of row r;
                # row r = p*TA + t in p-major order matches ch_sb layout
                tvf = Tv[:, :].bitcast(F32).rearrange("(p t) x -> p t x", p=P)
                chv = ch_sb[:].rearrange("p (t x) -> p t x", x=1)
                H = TA // 2
                nc.sync.dma_start(out=tvf[:, 0:H, 64:65], in_=chv[:, 0:H, :])
                nc.scalar.dma_start(out=tvf[:, H:TA, 64:65], in_=chv[:, H:TA, :])

            # ---------------- phase C (per-slot indirect gathers) ----------
            def c_compute(t):
                w = kts[t]
                o = off[t]
                g_all = gpools[0].tile([P, w * ES], FP16, tag="g")
                for s in range(w):
                    nc.gpsimd.indirect_dma_start(
                        out=g_all[:, s * ES:(s + 1) * ES], out_offset=None,
                        in_=Tv[:, :],
                        in_offset=bass.IndirectOffsetOnAxis(
                            ap=idxr_sb[:, o + s:o + s + 1], axis=0))
                cv = g_all[:].bitcast(F32).rearrange("p (c x) -> p c x", x=P)
                s_t = spool.tile([P, w], F32, tag="s")
                nc.vector.scalar_tensor_tensor(
                    out=s_t[:],
                    in0=cv[:, :, 64:65].rearrange("p c x -> p (c x)"),
                    scalar=strip[:, t:t + 1],
                    in1=bias_sb[:, o:o + w],
                    op0=ALU.mult, op1=ALU.add)
                nm = spool.tile([P, 1], F32, tag="nm")
                nc.vector.tensor_reduce(out=nm[:], in_=s_t[:],
                                        axis=mybir.AxisListType.X,
                                        op=ALU.max, negate=True)
                e = spool.tile([P, w], F32, tag="e")
                es = spool.tile([P, 1], F32, tag="es")
                nc.scalar.activation(out=e[:], in_=s_t[:], func=AF.Exp,
                                     bias=nm[:, 0:1], scale=1.0,
                                     accum_out=es[:, 0:1])
                rs = spool.tile([P, 1], F32, tag="rs")
                nc.vector.reciprocal(out=rs[:], in_=es[:])
                wt = spool.tile([P, w], F32, tag="wt")
                nc.vector.tensor_scalar_mul(out=wt[:], in0=e[:],
                                            scalar1=rs[:, 0:1])
                acc = accpool.tile([P, 134], FP16, tag="acc")
                nc.vector.tensor_scalar_mul(out=acc[:], in0=g_all[:, 0:134],
                                            scalar1=wt[:, 0:1])
                for s in range(1, w):
                    nc.vector.scalar_tensor_tensor(
                        out=acc[:], in0=g_all[:, s * ES:s * ES + 134],
                        scalar=wt[:, s:s + 1], in1=acc[:],
                        op0=ALU.mult, op1=ALU.add)
                tr = ps2.tile([P, P], FP16, tag="p2")
                nc.tensor.transpose(out=tr[:], in_=acc[:, 0:P],
                                    identity=ident16[:])
                aggVT = vtpool.tile([P, P], FP16, tag="aggVT")
                nc.scalar.activation(out=aggVT[:], in_=tr[:], func=AF.Copy)
                tr2 = ps2.tile([3, P], FP16, tag="p2", padded_shape=[P, P])
                nc.tensor.transpose(out=tr2[:], in_=acc[:, 130:133],
                                    identity=ident16[:])
                aggCT = vtpool.tile([3, P], FP16, tag="aggCT")
                nc.scalar.activation(out=aggCT[:], in_=tr2[:], func=AF.Copy)
                o_ps = ps1.tile([P, P], F32, tag="p1")
                nc.tensor.matmul(out=o_ps[:], lhsT=wo_sb[:], rhs=aggVT[:],
                                 start=True, stop=False)
                nc.tensor.matmul(out=o_ps[:], lhsT=wpor_sb[:], rhs=aggCT[:],
                                 start=False, stop=True)
                oT = opool.tile([P, P], FP16, tag="oT")
                nc.scalar.activation(out=oT[:], in_=o_ps[:], func=AF.Relu,
                                     bias=ob_sb[:, 0:1], scale=1.0)
                xo = rpool.tile([P, P], FP16, tag="xo")
                nc.sync.dma_start(out=xo[:], in_=xoT_in[:, t * P:(t + 1) * P])
                res = rpool.tile([P, P], F32, tag="res")
                nc.vector.tensor_tensor(out=res[:], in0=oT[:], in1=xo[:],
                                        op=ALU.add)
                nc.sync.dma_start(out=outT[:, t * P:(t + 1) * P], in_=res[:])

            with nc.named_scope("phaseC"):
                for t in range(TO):
                    c_compute(t)
            _stk.close()

    nc.finalize()
    return nc


def _prep(inputs):
    x = np.asarray(inputs["x"], np.float32)
    coords = np.asarray(inputs["coords"], np.float32)
    W_q = np.asarray(inputs["W_q"], np.float32)
    q_gamma = np.asarray(inputs["q_gamma"], np.float32)
    q_beta = np.asarray(inputs["q_beta"], np.float32)
    W_v = np.asarray(inputs["W_v"], np.float32)
    v_gamma = np.asarray(inputs["v_gamma"], np.float32)
    v_beta = np.asarray(inputs["v_beta"], np.float32)
    codebook = np.asarray(inputs["codebook"], np.float32)
    W_choice = np.asarray(inputs["W_choice"], np.float32)
    b_choice = np.asarray(inputs["b_choice"], np.float32)
    W_pos = np.asarray(inputs["W_pos"], np.float32)
    b_pos = np.asarray(inputs["b_pos"], np.float32)
    W_out = np.asarray(inputs["W_out"], np.float32)
    out_gamma = np.asarray(inputs["out_gamma"], np.float32)
    out_beta = np.asarray(inputs["out_beta"], np.float32)
    nbr_idx = np.asarray(inputs["nbr_idx"], np.int32)
    nbr_mask = np.asarray(inputs["nbr_mask"], np.int32)

    n = x.shape[0]
    assert n == N

    mask_pad = np.zeros((K, PAD), bool)
    mask_pad[:, :n] = nbr_mask > 0
    deg = mask_pad.sum(0)
    # neighbor window = owning core pair (sort-invariant: core r's table rows
    # always lie in window r//2)
    nbr_win = np.zeros((K, PAD), np.int8)
    nbr_win[:, :n] = (nbr_idx // WR).astype(np.int8)
    nwc = np.zeros((NW, PAD), np.int32)     # per-point per-window valid count
    for w in range(NW):
        nwc[w] = ((nbr_win == w) & mask_pad).sum(0)

    orders = []
    degs_sorted = np.empty((NCORE, NSH), np.int64)
    for r in range(NCORE):
        sl = slice(r * NSH, (r + 1) * NSH)
        o = np.lexsort((nwc[0][sl], -deg[sl]))
        orders.append(o)
        degs_sorted[r] = deg[sl][o]
    kts = tuple(int(max(1, degs_sorted[:, t * P:(t + 1) * P].max()))
                for t in range(TO))
    off = np.zeros(TO + 1, np.int64)
    off[1:] = np.cumsum(kts)
    SKT = int(off[-1])
    perm_full = np.concatenate([r * NSH + orders[r] for r in range(NCORE)])
    inv = np.empty(PAD, np.int64)
    inv[perm_full] = np.arange(PAD)

    xp = np.zeros((PAD, P), np.float32)
    xp[:n] = x
    xp2 = xp[perm_full]
    cp = np.zeros((PAD, 3), np.float32)
    cp[:n] = coords
    cp2 = cp[perm_full]

    xT16 = np.ascontiguousarray(xp2.T).astype(np.float16)
    c3g = np.ascontiguousarray(
        cp2.reshape(TA, P, 3).transpose(1, 0, 2).reshape(P, TA * 3)
    ).astype(np.float16)

    # ---- weight folds ----
    cb2 = float(np.dot(codebook, codebook))
    scb = np.sqrt(cb2).astype(np.float32)
    rep = P // VEC
    wq_flat = np.ascontiguousarray(W_q.transpose(1, 0, 2).reshape(P, K * VEC))
    wv = W_v * v_gamma[None, :]
    wqv = np.concatenate([wq_flat, wv], axis=1).astype(np.float16)
    vb = v_beta[None, :].astype(np.float16)
    wcp = codebook[:, None] * W_choice
    wcc = (scb * wcp.reshape(VEC, rep, P).sum(1)).astype(np.float16)
    bch = (scb * b_choice)[None, :].astype(np.float16)
    Wor = W_out.reshape(VEC, rep, P).sum(1)
    wo = (W_out * out_gamma[None, :]).astype(np.float16)
    wpor = ((W_pos @ Wor) * out_gamma[None, :]).astype(np.float16)
    ob = (out_beta + (b_pos @ Wor) * out_gamma)[:, None].astype(np.float32)

    # ---- phase A indices (valid-first, Z pads), in permuted column order ---
    idx_new = np.full((K, PAD), Z, np.int32)
    idx_new[:, :n] = np.where(nbr_mask > 0, inv[nbr_idx], Z).astype(np.int32)
    korder = np.argsort(~mask_pad, axis=0, kind="stable")
    idx_srt = np.take_along_axis(idx_new, korder, axis=0)
    # p-major Yf block id for table row r: (r % 128) * TA + r // 128
    blk = (idx_srt % P) * TA + idx_srt // P
    idxa = np.where(idx_srt != Z, blk.astype(np.int64) * K + korder,
                    ((Z % P) * TA + Z // P) * K).astype(np.int32)
    idxa = idxa[:, perm_full]
    idx_srt_p = idx_srt[:, perm_full]
    valid_p = np.take_along_axis(mask_pad, korder, axis=0)[:, perm_full]
    win_p = np.take_along_axis(nbr_win, korder, axis=0)[:, perm_full]
    deg_p = deg[perm_full]

    def packA(arr_core):
        out = np.empty((P, SKT), arr_core.dtype)
        a3 = arr_core.reshape(K, TO, P)
        for t in range(TO):
            out[:, off[t]:off[t + 1]] = a3[:kts[t], t, :].T
        return np.ascontiguousarray(out)

    # ---- phase C: slot-aligned Tv row ids + score bias ----
    Jcore = np.zeros((NCORE, TO, NW), np.int32)
    for r in range(NCORE):
        sl = slice(r * NSH, (r + 1) * NSH)
        cnt = np.stack([((win_p[:, sl] == w) & valid_p[:, sl]).sum(0)
                        for w in range(NW)])
        Jcore[r] = cnt.reshape(NW, TO, P).max(2).T
    Js = tuple(tuple(int(v) for v in Jcore[:, t, :].max(0))
               for t in range(TO))
    bias01 = np.where(valid_p, np.float32(0.0), np.float32(NEG)).astype(np.float32)

    in_maps = []
    shared = dict(xT16=xT16, c3g=c3g, wqv=wqv, vb=vb,
                  qg=q_gamma[:, None].astype(np.float32),
                  qb=q_beta[:, None].astype(np.float32),
                  wcc=wcc, bch=bch, wo=wo, wpor=wpor, ob=ob)
    for r in range(NCORE):
        sl = slice(r * NSH, (r + 1) * NSH)
        m = dict(shared)
        m["idxa_p"] = packA(idxa[:, sl])
        m["idxr_p"] = packA(idx_srt_p[:, sl])
        m["bias_p"] = packA(bias01[:, sl])
        m["xoT"] = np.ascontiguousarray(xp2[sl].T).astype(np.float16)
        in_maps.append(m)
    return in_maps, kts, Js, orders


def prepare(inputs):
    in_maps, kts, Js, orders = _prep(inputs)
    key = (kts, Js)
    if _CACHE.get("key") != key:
        _CACHE["nc"] = _build_nc(kts, Js)
        _CACHE["key"] = key
    return _CACHE["nc"], in_maps, orders


def assemble(results, orders):
    out = np.empty((NCORE * NSH, P), np.float32)
    for r in range(NCORE):
        out[r * NSH + orders[r]] = results[r]["outT"].T
    return np.ascontiguousarray(out[:N])


def kernel(**inputs):
    nc, in_maps, orders = prepare(inputs)
    res = run_bass_kernel_spmd(nc, in_maps, list(range(NCORE)))
    return assemble(res.results, orders)


if __name__ == "__main__":
    rng = np.random.default_rng(0)
    ins = dict(
        x=rng.standard_normal((N, P)).astype(np.float32),
        coords=(rng.random((N, 3)) * 100).astype(np.float32),
        W_q=rng.standard_normal((K, P, VEC)).astype(np.float32) * (P * K) ** -0.5,
        q_gamma=np.ones(VEC, np.float32), q_beta=np.zeros(VEC, np.float32),
        W_v=rng.standard_normal((P, P)).astype(np.float32) * P ** -0.5,
        v_gamma=np.ones(P, np.float32), v_beta=np.zeros(P, np.float32),
        codebook=rng.standard_normal(P).astype(np.float32) * 0.1,
        W_choice=rng.standard_normal((P, P)).astype(np.float32) * P ** -0.5,
        b_choice=np.zeros(P, np.float32),
        W_pos=rng.standard_normal((3, VEC)).astype(np.float32) * 3 ** -0.5,
        b_pos=np.zeros(VEC, np.float32),
        W_out=rng.standard_normal((P, P)).astype(np.float32) * P ** -0.5,
        out_gamma=np.ones(P, np.float32), out_beta=np.zeros(P, np.float32),
        nbr_idx=rng.integers(0, N, (K, N)).astype(np.int32),
        nbr_mask=rng.integers(0, 2, (K, N)).astype(np.int32),
    )
    out = kernel(**ins)
    print("kernel output", out.shape, out.dtype)


# revision 22
# speedup vs baseline: 1.3777x; 1.3777x over previous
"""Trainium2 Bass kernel for nn_DiscreteQKTRBlock (sparse 3x3x3 neighborhood
attention with a discrete codebook).

Strategy (data-parallel over points, 8 cores, replicated tables):

Algebraic collapse: dq[i] = codebook * choice[i], so the per-offset score is
s[k,i] = ||codebook||^2 * choice[i] * choice[nbr[k,i]] - a scalar product.

Phases per core (own points NSH=12544 = 98 tiles; tables PAD=100352 rows):
  Y      build Yf[(j,k)] = x[j] @ Wq_k  (fp16, desc-optimized p-major layout)
  A      per own tile, per valid slot: 128-row indirect gather from Yf with
         CCE-add accumulation -> q, then BN/relu + choice head -> strip.
         (HW indirect DMA processes exactly one index per partition.)
  TV     (overlapped under A) build Tv[j] = [v fp16 128 | choice f32 | xyz
         fp16] in 512B rows; positional encoding is NOT applied: coords ride
         in the row, the softmax-weighted sum aggregates them (sum w = 1), and
         W_pos / b_pos are folded into the output projection / bias.
  AG     transpose strip to point-major, AllGather choice, patch the choice
         f32 column into all Tv rows with one strided write.
  C      windowed int16 dma_gather of Tv rows: the 100352-row table is
         addressed in four <32768-row windows; each point's valid slots are
         compacted per (tile, window) in per-partition SCRAMBLED order (the
         softmax and the weighted sum are order-agnostic per partition; the
         score bias is host-staged in the same order and choice rides in the
         gathered row, so scrambling is benign).  One dma_gather call per
         (3-tile group, window) replaces ~40 per-slot SWDGE calls.
         Then masked softmax in f32, fp16 weighted aggregation (values +
         coords together), folded output projection, relu, residual.
"""
import sys
sys.path.insert(0, "/opt/trn_rl_repo")
import numpy as np
import ml_dtypes

from concourse import bass, bacc, mybir
import concourse.tile as tile
from concourse.bass_utils import run_bass_kernel_spmd
from concourse.masks import make_identity

F32 = mybir.dt.float32
FP16 = mybir.dt.float16
I32 = mybir.dt.int32
I16 = mybir.dt.int16
AF = mybir.ActivationFunctionType
ALU = mybir.AluOpType

N = 100000
P = 128
VEC = 16
K = 27
NEG = -1e9
NCORE = 8
NSH = 12544                # points per core (98 tiles of 128)
TO = NSH // P              # 98 own tiles
PAD = NCORE * NSH          # 100352 table rows
TA = PAD // P              # 784 table tiles
Z = N                      # zero-row index for masked/padded neighbors
NW = 2                     # dma_gather windows over the table
WR = PAD // NW             # 50176 rows per window (signed-int16 centered base)
WC = WR // 2               # centered base offset within a window
ES = 256                   # Tv row elems (fp16): v 128 | choice f32 | xyz | pad
G = 8                      # build tiles per group
NG = TA // G
GC = 2                     # own tiles per C gather group

_CACHE = {}


def _build_nc(kts, Js):
    off = [0]
    for w in kts:
        off.append(off[-1] + int(w))
    SKT = off[-1]
    # C-phase position bookkeeping (all build-time constants)
    posoff = [0]            # per tile: start of its positions
    for t in range(TO):
        posoff.append(posoff[-1] + sum(Js[t]))
    TOTPOS = posoff[-1]
    groups = [list(range(g, min(g + GC, TO))) for g in range(0, TO, GC)]
    # per (group, window): column count and idxC offset (in positions)
    call_cols = {}
    cum = 0
    for gi, gts in enumerate(groups):
        for w in range(NW):
            c = sum(Js[t][w] for t in gts)
            if c > 0:
                c += 1          # trailing all-pad column (negative-trim guard)
            call_cols[(gi, w)] = (cum, c)
            cum += c
    NCOLS = cum

    nc = bacc.Bacc(num_devices=NCORE, dynamic_dma_scratch_size=32768)

    # ---------------- inputs ----------------
    xT16 = nc.declare_dram_parameter("xT16", [P, PAD], FP16, isOutput=False)
    c3g_in = nc.declare_dram_parameter("c3g", [P, TA * 3], FP16, isOutput=False)
    wqv_in = nc.declare_dram_parameter("wqv", [P, 560], FP16, isOutput=False)
    vb_in = nc.declare_dram_parameter("vb", [1, P], FP16, isOutput=False)
    qg_in = nc.declare_dram_parameter("qg", [VEC, 1], F32, isOutput=False)
    qb_in = nc.declare_dram_parameter("qb", [VEC, 1], F32, isOutput=False)
    wcc_in = nc.declare_dram_parameter("wcc", [VEC, P], FP16, isOutput=False)
    bch_in = nc.declare_dram_parameter("bch", [1, P], FP16, isOutput=False)
    wo_in = nc.declare_dram_parameter("wo", [P, P], FP16, isOutput=False)
    wpor_in = nc.declare_dram_parameter("wpor", [3, P], FP16, isOutput=False)
    ob_in = nc.declare_dram_parameter("ob", [P, 1], F32, isOutput=False)
    idxa_in = nc.declare_dram_parameter("idxa_p", [P, SKT], I32, isOutput=False)
    idxr_in = nc.declare_dram_parameter("idxr_p", [P, SKT], I32, isOutput=False)
    bias_in = nc.declare_dram_parameter("bias_p", [P, SKT], F32,
                                        isOutput=False)
    xoT_in = nc.declare_dram_parameter("xoT", [P, NSH], FP16, isOutput=False)

    outT = nc.declare_dram_parameter("outT", [P, NSH], F32, isOutput=True)

    with tile.TileContext(nc) as tc:
        with tc.tile_pool(name="const", bufs=1) as cpool, \
             tc.tile_pool(name="dram", bufs=1, space="DRAM") as dpool:

            # ---------------- resident constants ----------------
            wqv_sb = cpool.tile([P, 560], FP16)
            nc.scalar.dma_start(out=wqv_sb[:], in_=wqv_in[:, :])
            vb_sb = cpool.tile([1, P], FP16)
            nc.scalar.dma_start(out=vb_sb[:], in_=vb_in[:, :])
            qg_sb = cpool.tile([VEC, 1], F32)
            nc.scalar.dma_start(out=qg_sb[:], in_=qg_in[:, :])
            qb_sb = cpool.tile([VEC, 1], F32)
            nc.scalar.dma_start(out=qb_sb[:], in_=qb_in[:, :])
            wcc_sb = cpool.tile([VEC, P], FP16)
            nc.scalar.dma_start(out=wcc_sb[:], in_=wcc_in[:, :])
            bch_sb = cpool.tile([1, P], FP16)
            nc.scalar.dma_start(out=bch_sb[:], in_=bch_in[:, :])
            wo_sb = cpool.tile([P, P], FP16)
            nc.scalar.dma_start(out=wo_sb[:], in_=wo_in[:, :])
            wpor_sb = cpool.tile([3, P], FP16)
            nc.scalar.dma_start(out=wpor_sb[:], in_=wpor_in[:, :])
            ob_sb = cpool.tile([P, 1], F32)
            nc.scalar.dma_start(out=ob_sb[:], in_=ob_in[:, :])
            idxa_sb = cpool.tile([P, SKT], I32)
            nc.sync.dma_start(out=idxa_sb[:], in_=idxa_in[:, :])
            idxr_sb = cpool.tile([P, SKT], I32)
            nc.sync.dma_start(out=idxr_sb[:], in_=idxr_in[:, :])
            bias_sb = cpool.tile([P, SKT], F32)
            nc.sync.dma_start(out=bias_sb[:], in_=bias_in[:, :])
            c3g_sb = cpool.tile([P, TA * 3], FP16)
            nc.sync.dma_start(out=c3g_sb[:], in_=c3g_in[:, :])

            ident16 = cpool.tile([P, P], FP16)
            make_identity(nc, ident16[:])
            ident32 = cpool.tile([P, P], F32)
            make_identity(nc, ident32[:])
            ones16 = cpool.tile([1, P], FP16)
            nc.vector.memset(ones16[:], 1.0)

            strip = cpool.tile([P, TO], F32)
            st_sb = cpool.tile([P, P], F32)
            ch_sb = cpool.tile([P, TA], F32)     # allgathered choice, p-major

            # ---------------- DRAM tables ----------------
            Yf = dpool.tile([PAD * K, VEC], FP16)
            Tv = dpool.tile([PAD, ES], FP16)
            cc_in = dpool.tile([NSH, 1], F32)
            AGout = dpool.tile([NCORE, NSH, 1], F32, addr_space="Shared")

            from contextlib import ExitStack
            _stk = ExitStack()
            xpool = _stk.enter_context(tc.tile_pool(name="b_x", bufs=3))
            ypool = _stk.enter_context(tc.tile_pool(name="b_y", bufs=2))
            tpool = _stk.enter_context(tc.tile_pool(name="b_t", bufs=2))
            ps1 = _stk.enter_context(tc.tile_pool(name="ps1", bufs=2, space="PSUM"))
            ps2 = _stk.enter_context(tc.tile_pool(name="ps2", bufs=4, space="PSUM"))
            apool = _stk.enter_context(tc.tile_pool(name="a_g", bufs=3))
            qfpool = _stk.enter_context(tc.tile_pool(name="a_qf", bufs=2))
            scrpool = _stk.enter_context(tc.tile_pool(name="a_scr", bufs=2))
            gpools = [_stk.enter_context(tc.tile_pool(name=f"c_g{w}", bufs=2))
                      for w in range(NW)]
            spool = _stk.enter_context(tc.tile_pool(name="c_s", bufs=3))
            accpool = _stk.enter_context(tc.tile_pool(name="c_acc", bufs=2))
            vtpool = _stk.enter_context(tc.tile_pool(name="c_vt", bufs=2))
            opool = _stk.enter_context(tc.tile_pool(name="c_o", bufs=2))
            rpool = _stk.enter_context(tc.tile_pool(name="c_r", bufs=3))

            # ---------------- phase Y: Yf = x @ Wq (p-major blocks) --------
            with nc.named_scope("phaseY"):
                for g in range(NG):
                    xg = xpool.tile([P, G * P], FP16, tag="xg")
                    nc.sync.dma_start(out=xg[:],
                                      in_=xT16[:, g * G * P:(g + 1) * G * P])
                    y8 = ypool.tile([P, G * 432], FP16, tag="y8")
                    for i in range(G):
                        y_ps = ps1.tile([P, 432], F32, tag="p1")
                        nc.tensor.matmul(out=y_ps[:],
                                         lhsT=xg[:, i * P:(i + 1) * P],
                                         rhs=wqv_sb[:, 0:432],
                                         start=True, stop=True)
                        ysl = y8[:, i * 432:(i + 1) * 432]
                        if i % 2 == 0:
                            nc.vector.tensor_copy(out=ysl, in_=y_ps[:])
                        else:
                            nc.scalar.activation(out=ysl, in_=y_ps[:],
                                                 func=AF.Copy)
                    # partition p holds Y blocks for table rows (t*128+p),
                    # stored p-major: block index p*TA + t  -> one descriptor
                    # per partition per group
                    ydst = bass.AP(Yf.tensor, Yf[:].offset + g * G * 432,
                                   [(TA * 432, P), (1, G * 432)])
                    nc.scalar.dma_start(out=ydst, in_=y8[:])

            # ---------------- phase TV: value table (overlaps phase A) -----
            def emit_tv_group(g):
                if True:
                    xg = xpool.tile([P, G * P], FP16, tag="xg")
                    nc.sync.dma_start(out=xg[:],
                                      in_=xT16[:, g * G * P:(g + 1) * G * P])
                    t8 = tpool.tile([P, G * ES], FP16, tag="t8")
                    for i in range(G):
                        v_ps = ps2.tile([P, P], F32, tag="p2")
                        nc.tensor.matmul(out=v_ps[:],
                                         lhsT=xg[:, i * P:(i + 1) * P],
                                         rhs=wqv_sb[:, 432:560],
                                         start=True, stop=False)
                        nc.tensor.matmul(out=v_ps[:], lhsT=ones16[:],
                                         rhs=vb_sb[:], start=False, stop=True)
                        tsl = t8[:, i * ES:i * ES + P]
                        if i % 2 == 0:
                            nc.scalar.activation(out=tsl, in_=v_ps[:],
                                                 func=AF.Relu)
                        else:
                            nc.vector.tensor_scalar_max(out=tsl, in0=v_ps[:],
                                                        scalar1=0.0)
                        # coords into row elems 130..132
                        gi = g * G + i
                        nc.vector.tensor_copy(
                            out=t8[:, i * ES + 130:i * ES + 133],
                            in_=c3g_sb[:, gi * 3:(gi + 1) * 3])
                    tdst = bass.AP(Tv.tensor, Tv[:].offset + g * G * P * ES,
                                   [(ES, P), (ES * P, G), (1, ES)])
                    nc.sync.dma_start(out=tdst, in_=t8[:])

            # ---------------- phase A: q + choice (per-slot gathers) -------
            def emit_a_tile(t):
                if True:
                    w = kts[t]
                    ya = apool.tile([P, w * VEC], FP16, tag="qacc")
                    for s in range(w):
                        nc.gpsimd.indirect_dma_start(
                            out=ya[:, s * VEC:(s + 1) * VEC], out_offset=None,
                            in_=Yf[:, :],
                            in_offset=bass.IndirectOffsetOnAxis(
                                ap=idxa_sb[:, off[t] + s:off[t] + s + 1],
                                axis=0))
                    cur = w
                    b = 0
                    while cur > 1:
                        if cur % 2 == 1:
                            nc.vector.tensor_tensor(
                                out=ya[:, b:b + VEC], in0=ya[:, b:b + VEC],
                                in1=ya[:, (cur - 1) * VEC:cur * VEC],
                                op=ALU.add)
                            cur -= 1
                            if cur == 1:
                                break
                        h = cur // 2
                        nc.vector.tensor_tensor(
                            out=ya[:, b:b + h * VEC], in0=ya[:, b:b + h * VEC],
                            in1=ya[:, h * VEC:2 * h * VEC], op=ALU.add)
                        cur = h
                    q_ps = ps2.tile([VEC, P], FP16, tag="p2",
                                    padded_shape=[P, P])
                    nc.tensor.transpose(out=q_ps[:], in_=ya[:, 0:VEC],
                                        identity=ident16[:])
                    qf = qfpool.tile([VEC, P], FP16, tag="qf")
                    nc.scalar.activation(out=qf[:], in_=q_ps[:], func=AF.Relu,
                                         bias=qb_sb[:, 0:1],
                                         scale=qg_sb[:, 0:1])
                    t_ps = ps1.tile([P, P], F32, tag="p1")
                    nc.tensor.matmul(out=t_ps[:], lhsT=qf[:], rhs=wcc_sb[:],
                                     start=True, stop=False)
                    nc.tensor.matmul(out=t_ps[:], lhsT=ones16[:],
                                     rhs=bch_sb[:], start=False, stop=True)
                    scr = scrpool.tile([P, P], F32, tag="scr")
                    nc.scalar.activation(out=scr[:], in_=t_ps[:], func=AF.Relu,
                                         accum_out=strip[:, t:t + 1])

            def emit_atv():
                for i in range(max(NG, TO)):
                    if i < NG:
                        emit_tv_group(i)
                    if i < TO:
                        emit_a_tile(i)

            # interleave TV groups and A tiles so the in-order PE/ACT/DVE
            # queues don't head-of-line block TV behind A's gather waits
            with nc.named_scope("phaseATV"):
                emit_atv()

            # ---------------- AllGather choice + patch into Tv -------------
            with nc.named_scope("gather_choice"):
                st_ps = ps1.tile([P, P], F32, tag="p1")
                nc.tensor.transpose(out=st_ps[0:TO, :], in_=strip[:],
                                    identity=ident32[:])
                nc.vector.tensor_copy(out=st_sb[0:TO, :], in_=st_ps[0:TO, :])
                ccdst = bass.AP(cc_in.tensor, cc_in[:].offset, [(P, TO), (1, P)])
                nc.sync.dma_start(out=ccdst, in_=st_sb[0:TO, :])
                nc.gpsimd.collective_compute(
                    "AllGather", ALU.bypass,
                    replica_groups=[list(range(NCORE))],
                    ins=[cc_in.opt()], outs=[AGout.opt()])
                # load p-major: ch_sb[p, t] = choice[table row p*TA + t]
                agf = AGout[:, :, :].rearrange("r n v -> (r n v)")
                nc.sync.dma_start(
                    out=ch_sb[:], in_=agf.rearrange("(p t) -> p t", p=P))
                # patch: choice f32 lives at fp16-elems [128,130) of row r;
                # row r = p*TA + t in p-major order matches ch_sb layout
                tvf = Tv[:, :].bitcast(F32).rearrange("(p t) x -> p t x", p=P)
                chv = ch_sb[:].rearrange("p (t x) -> p t x", x=1)
                H = TA // 2
                nc.sync.dma_start(out=tvf[:, 0:H, 64:65], in_=chv[:, 0:H, :])
                nc.scalar.dma_start(out=tvf[:, H:TA, 64:65], in_=chv[:, H:TA, :])

            # ---------------- phase C (per-slot indirect gathers) ----------
            def c_compute(t):
                w = kts[t]
                o = off[t]
                g_all = gpools[0].tile([P, w * ES], FP16, tag="g")
                for s in range(w):
                    nc.gpsimd.indirect_dma_start(
                        out=g_all[:, s * ES:(s + 1) * ES], out_offset=None,
                        in_=Tv[:, :],
                        in_offset=bass.IndirectOffsetOnAxis(
                            ap=idxr_sb[:, o + s:o + s + 1], axis=0))
                cv = g_all[:].bitcast(F32).rearrange("p (c x) -> p c x", x=P)
                s_t = spool.tile([P, w], F32, tag="s")
                nc.vector.scalar_tensor_tensor(
                    out=s_t[:],
                    in0=cv[:, :, 64:65].rearrange("p c x -> p (c x)"),
                    scalar=strip[:, t:t + 1],
                    in1=bias_sb[:, o:o + w],
                    op0=ALU.mult, op1=ALU.add)
                nm = spool.tile([P, 1], F32, tag="nm")
                nc.vector.tensor_reduce(out=nm[:], in_=s_t[:],
                                        axis=mybir.AxisListType.X,
                                        op=ALU.max, negate=True)
                e = spool.tile([P, w], F32, tag="e")
                es = spool.tile([P, 1], F32, tag="es")
                nc.scalar.activation(out=e[:], in_=s_t[:], func=AF.Exp,
                                     bias=nm[:, 0:1], scale=1.0,
                                     accum_out=es[:, 0:1])
                rs = spool.tile([P, 1], F32, tag="rs")
                nc.vector.reciprocal(out=rs[:], in_=es[:])
                wt = spool.tile([P, w], F32, tag="wt")
                nc.vector.tensor_scalar_mul(out=wt[:], in0=e[:],
                                            scalar1=rs[:, 0:1])
                acc = accpool.tile([P, 134], FP16, tag="acc")
                nc.vector.tensor_scalar_mul(out=acc[:], in0=g_all[:, 0:134],
                                            scalar1=wt[:, 0:1])
                for s in range(1, w):
                    nc.vector.scalar_tensor_tensor(
                        out=acc[:], in0=g_all[:, s * ES:s * ES + 134],
                        scalar=wt[:, s:s + 1], in1=acc[:],
                        op0=ALU.mult, op1=ALU.add)
                tr = ps2.tile([P, P], FP16, tag="p2")
                nc.tensor.transpose(out=tr[:], in_=acc[:, 0:P],
                                    identity=ident16[:])
                aggVT = vtpool.tile([P, P], FP16, tag="aggVT")
                nc.scalar.activation(out=aggVT[:], in_=tr[:], func=AF.Copy)
                tr2 = ps2.tile([3, P], FP16, tag="p2", padded_shape=[P, P])
                nc.tensor.transpose(out=tr2[:], in_=acc[:, 130:133],
                                    identity=ident16[:])
                aggCT = vtpool.tile([3, P], FP16, tag="aggCT")
                nc.scalar.activation(out=aggCT[:], in_=tr2[:], func=AF.Copy)
                o_ps = ps1.tile([P, P], F32, tag="p1")
                nc.tensor.matmul(out=o_ps[:], lhsT=wo_sb[:], rhs=aggVT[:],
                                 start=True, stop=False)
                nc.tensor.matmul(out=o_ps[:], lhsT=wpor_sb[:], rhs=aggCT[:],
                                 start=False, stop=True)
                oT = opool.tile([P, P], FP16, tag="oT")
                nc.scalar.activation(out=oT[:], in_=o_ps[:], func=AF.Relu,
                                     bias=ob_sb[:, 0:1], scale=1.0)
                xo = rpool.tile([P, P], FP16, tag="xo")
                nc.sync.dma_start(out=xo[:], in_=xoT_in[:, t * P:(t + 1) * P])
                res = rpool.tile([P, P], F32, tag="res")
                nc.vector.tensor_tensor(out=res[:], in0=oT[:], in1=xo[:],
                                        op=ALU.add)
                nc.sync.dma_start(out=outT[:, t * P:(t + 1) * P], in_=res[:])

            with nc.named_scope("phaseC"):
                for t in range(TO):
                    c_compute(t)
            _stk.close()

    nc.finalize()
    return nc


def _prep(inputs):
    x = np.asarray(inputs["x"], np.float32)
    coords = np.asarray(inputs["coords"], np.float32)
    W_q = np.asarray(inputs["W_q"], np.float32)
    q_gamma = np.asarray(inputs["q_gamma"], np.float32)
    q_beta = np.asarray(inputs["q_beta"], np.float32)
    W_v = np.asarray(inputs["W_v"], np.float32)
    v_gamma = np.asarray(inputs["v_gamma"], np.float32)
    v_beta = np.asarray(inputs["v_beta"], np.float32)
    codebook = np.asarray(inputs["codebook"], np.float32)
    W_choice = np.asarray(inputs["W_choice"], np.float32)
    b_choice = np.asarray(inputs["b_choice"], np.float32)
    W_pos = np.asarray(inputs["W_pos"], np.float32)
    b_pos = np.asarray(inputs["b_pos"], np.float32)
    W_out = np.asarray(inputs["W_out"], np.float32)
    out_gamma = np.asarray(inputs["out_gamma"], np.float32)
    out_beta = np.asarray(inputs["out_beta"], np.float32)
    nbr_idx = np.asarray(inputs["nbr_idx"], np.int32)
    nbr_mask = np.asarray(inputs["nbr_mask"], np.int32)

    n = x.shape[0]
    assert n == N

    mask_pad = np.zeros((K, PAD), bool)
    mask_pad[:, :n] = nbr_mask > 0
    deg = mask_pad.sum(0)
    # neighbor window = owning core pair (sort-invariant: core r's table rows
    # always lie in window r//2)
    nbr_win = np.zeros((K, PAD), np.int8)
    nbr_win[:, :n] = (nbr_idx // WR).astype(np.int8)
    nwc = np.zeros((NW, PAD), np.int32)     # per-point per-window valid count
    for w in range(NW):
        nwc[w] = ((nbr_win == w) & mask_pad).sum(0)

    orders = []
    degs_sorted = np.empty((NCORE, NSH), np.int64)
    for r in range(NCORE):
        sl = slice(r * NSH, (r + 1) * NSH)
        o = np.lexsort((nwc[0][sl], -deg[sl]))
        orders.append(o)
        degs_sorted[r] = deg[sl][o]
    kts = tuple(int(max(1, degs_sorted[:, t * P:(t + 1) * P].max()))
                for t in range(TO))
    off = np.zeros(TO + 1, np.int64)
    off[1:] = np.cumsum(kts)
    SKT = int(off[-1])
    perm_full = np.concatenate([r * NSH + orders[r] for r in range(NCORE)])
    inv = np.empty(PAD, np.int64)
    inv[perm_full] = np.arange(PAD)

    xp = np.zeros((PAD, P), np.float32)
    xp[:n] = x
    xp2 = xp[perm_full]
    cp = np.zeros((PAD, 3), np.float32)
    cp[:n] = coords
    cp2 = cp[perm_full]

    xT16 = np.ascontiguousarray(xp2.T).astype(np.float16)
    c3g = np.ascontiguousarray(
        cp2.reshape(TA, P, 3).transpose(1, 0, 2).reshape(P, TA * 3)
    ).astype(np.float16)

    # ---- weight folds ----
    cb2 = float(np.dot(codebook, codebook))
    scb = np.sqrt(cb2).astype(np.float32)
    rep = P // VEC
    wq_flat = np.ascontiguousarray(W_q.transpose(1, 0, 2).reshape(P, K * VEC))
    wv = W_v * v_gamma[None, :]
    wqv = np.concatenate([wq_flat, wv], axis=1).astype(np.float16)
    vb = v_beta[None, :].astype(np.float16)
    wcp = codebook[:, None] * W_choice
    wcc = (scb * wcp.reshape(VEC, rep, P).sum(1)).astype(np.float16)
    bch = (scb * b_choice)[None, :].astype(np.float16)
    Wor = W_out.reshape(VEC, rep, P).sum(1)
    wo = (W_out * out_gamma[None, :]).astype(np.float16)
    wpor = ((W_pos @ Wor) * out_gamma[None, :]).astype(np.float16)
    ob = (out_beta + (b_pos @ Wor) * out_gamma)[:, None].astype(np.float32)

    # ---- phase A indices (valid-first, Z pads), in permuted column order ---
    idx_new = np.full((K, PAD), Z, np.int32)
    idx_new[:, :n] = np.where(nbr_mask > 0, inv[nbr_idx], Z).astype(np.int32)
    korder = np.argsort(~mask_pad, axis=0, kind="stable")
    idx_srt = np.take_along_axis(idx_new, korder, axis=0)
    # p-major Yf block id for table row r: (r % 128) * TA + r // 128
    blk = (idx_srt % P) * TA + idx_srt // P
    idxa = np.where(idx_srt != Z, blk.astype(np.int64) * K + korder,
                    ((Z % P) * TA + Z // P) * K).astype(np.int32)
    idxa = idxa[:, perm_full]
    idx_srt_p = idx_srt[:, perm_full]
    valid_p = np.take_along_axis(mask_pad, korder, axis=0)[:, perm_full]
    win_p = np.take_along_axis(nbr_win, korder, axis=0)[:, perm_full]
    deg_p = deg[perm_full]

    def packA(arr_core):
        out = np.empty((P, SKT), arr_core.dtype)
        a3 = arr_core.reshape(K, TO, P)
        for t in range(TO):
            out[:, off[t]:off[t + 1]] = a3[:kts[t], t, :].T
        return np.ascontiguousarray(out)

    # ---- phase C: slot-aligned Tv row ids + score bias ----
    Jcore = np.zeros((NCORE, TO, NW), np.int32)
    for r in range(NCORE):
        sl = slice(r * NSH, (r + 1) * NSH)
        cnt = np.stack([((win_p[:, sl] == w) & valid_p[:, sl]).sum(0)
                        for w in range(NW)])
        Jcore[r] = cnt.reshape(NW, TO, P).max(2).T
    Js = tuple(tuple(int(v) for v in Jcore[:, t, :].max(0))
               for t in range(TO))
    bias01 = np.where(valid_p, np.float32(0.0), np.float32(NEG)).astype(np.float32)

    in_maps = []
    shared = dict(xT16=xT16, c3g=c3g, wqv=wqv, vb=vb,
                  qg=q_gamma[:, None].astype(np.float32),
                  qb=q_beta[:, None].astype(np.float32),
                  wcc=wcc, bch=bch, wo=wo, wpor=wpor, ob=ob)
    for r in range(NCORE):
        sl = slice(r * NSH, (r + 1) * NSH)
        m = dict(shared)
        m["idxa_p"] = packA(idxa[:, sl])
        m["idxr_p"] = packA(idx_srt_p[:, sl])
        m["bias_p"] = packA(bias01[:, sl])
        m["xoT"] = np.ascontiguousarray(xp2[sl].T).astype(np.float16)
        in_maps.append(m)
    return in_maps, kts, Js, orders


def prepare(inputs):
    in_maps, kts, Js, orders = _prep(inputs)
    key = (kts, Js)
    if _CACHE.get("key") != key:
        _CACHE["nc"] = _build_nc(kts, Js)
        _CACHE["key"] = key
    return _CACHE["nc"], in_maps, orders


def assemble(results, orders):
    out = np.empty((NCORE * NSH, P), np.float32)
    for r in range(NCORE):
        out[r * NSH + orders[r]] = results[r]["outT"].T
    return np.ascontiguousarray(out[:N])


def kernel(**inputs):
    nc, in_maps, orders = prepare(inputs)
    res = run_bass_kernel_spmd(nc, in_maps, list(range(NCORE)))
    return assemble(res.results, orders)


if __name__ == "__main__":
    rng = np.random.default_rng(0)
    ins = dict(
        x=rng.standard_normal((N, P)).astype(np.float32),
        coords=(rng.random((N, 3)) * 100).astype(np.float32),
        W_q=rng.standard_normal((K, P, VEC)).astype(np.float32) * (P * K) ** -0.5,
        q_gamma=np.ones(VEC, np.float32), q_beta=np.zeros(VEC, np.float32),
        W_v=rng.standard_normal((P, P)).astype(np.float32) * P ** -0.5,
        v_gamma=np.ones(P, np.float32), v_beta=np.zeros(P, np.float32),
        codebook=rng.standard_normal(P).astype(np.float32) * 0.1,
        W_choice=rng.standard_normal((P, P)).astype(np.float32) * P ** -0.5,
        b_choice=np.zeros(P, np.float32),
        W_pos=rng.standard_normal((3, VEC)).astype(np.float32) * 3 ** -0.5,
        b_pos=np.zeros(VEC, np.float32),
        W_out=rng.standard_normal((P, P)).astype(np.float32) * P ** -0.5,
        out_gamma=np.ones(P, np.float32), out_beta=np.zeros(P, np.float32),
        nbr_idx=rng.integers(0, N, (K, N)).astype(np.int32),
        nbr_mask=rng.integers(0, 2, (K, N)).astype(np.int32),
    )
    out = kernel(**ins)
    print("kernel output", out.shape, out.dtype)
